# revision 1
# baseline (speedup 1.0000x reference)
"""Trainium2 Bass kernel for a 12-layer GRU LM (nn_CudaGRULM).

Model: h = emb[x]; 12x { residual + Wout @ GRU(Win @ LN(h)) }; LN; logits = h @ emb.T
Shapes: V=256, D=512, DEPTH=12, DI=512, B=16, T=2048.

Strategy:
 - Data-parallel over batch: 8 cores x B_local=2. No collectives; each core
   computes its batch shard end-to-end and returns its logits shard.
 - Everything on device lives in "T-layout": feature dim on partitions
   (4 chunks of 128), tokens along the free dim (col = t*B_local + b).
 - Host-side exact algebra: xzr = LN(h) @ (W_zr@Win).T + b  ==> fused weight
   W_zr@Win computed in float64 on host; LN gamma/beta folded into the fused
   weights/biases; embedding gather done as one-hot matmul on device.
 - GRU scan: recurrent weights stationary on the PE (LDWEIGHTS-bound),
   state is a [128, B_local] column pair read/written in the hsT ring tile.
"""

import os
from contextlib import ExitStack

import numpy as np

import concourse.bass as bass
import concourse.bacc as bacc
import concourse.tile as tile
from concourse import mybir
from concourse.bass_utils import run_bass_kernel_spmd

FP = mybir.dt.float32
BF = mybir.dt.bfloat16
AF = mybir.ActivationFunctionType
ALU = mybir.AluOpType


class Cfg:
    def __init__(self, V=256, D=512, DEPTH=12, DI=512, B=16, T=2048,
                 n_cores=8, S=256, U=16, EPS=1e-5, bf16_scan=True):
        self.bf16_scan = bf16_scan
        self.V, self.D, self.DEPTH, self.DI, self.B, self.T = V, D, DEPTH, DI, B, T
        self.n_cores = n_cores
        self.BL = B // n_cores          # local batch per core
        self.NTOK = T * self.BL         # tokens per core
        self.S = S                      # scan steps per chunk
        self.U = U                      # scan unroll inside For_i
        self.NCHUNK = T // S
        self.CC = S * self.BL           # chunk cols
        self.EPS = EPS
        self.KD = D // 128              # 4  (feature chunks)
        self.KV = V // 128              # 2  (vocab chunks)
        self.MZR = 2 * DI // 128        # 8  (zr gate chunks)
        self.MH = DI // 128             # 4  (h gate chunks)
        assert D == DI, "layout assumes D == DI"
        assert self.S % self.U == 0 and T % S == 0 and B % n_cores == 0


def build_kernel(ctx: ExitStack, tc: "tile.TileContext", outs, ins, cfg: Cfg):
    nc = tc.nc
    c = cfg
    KD, KV, MZR, MH, BL, CC, S, U = c.KD, c.KV, c.MZR, c.MH, c.BL, c.CC, c.S, c.U

    logits = outs["logits"]

    persist = ctx.enter_context(tc.tile_pool(name="persist", bufs=1))
    wpool = ctx.enter_context(tc.tile_pool(name="wpool", bufs=1))
    sb = ctx.enter_context(tc.tile_pool(name="sb", bufs=2))
    ps_scan = ctx.enter_context(tc.tile_pool(name="ps_scan", bufs=2, space="PSUM"))
    ps_proj = ctx.enter_context(tc.tile_pool(name="ps_proj", bufs=2, space="PSUM"))
    ps_bc = ctx.enter_context(tc.tile_pool(name="ps_bc", bufs=2, space="PSUM"))

    # ---- persistent state ----
    h_sb = persist.tile([128, KD, c.NTOK], FP)            # residual stream (T-layout)
    hsT = persist.tile([128, KD, (S + 1) * BL], FP)       # scan state/output ring
    SDT = BF if c.bf16_scan else FP
    xzrT = persist.tile([128, MZR, CC], FP)               # per-chunk input proj (zr)
    xhT = persist.tile([128, MH, CC], FP)                 # per-chunk input proj (h)
    hn_sb = persist.tile([128, KD, CC], FP)               # normalized chunk
    sq_sb = persist.tile([128, KD, CC], FP)               # squared chunk (LN stats)

    # ---- constants ----
    iota2 = persist.tile([128, KV], FP)
    nc.sync.dma_start(iota2[:], ins["iota2"][:])
    ones_col = persist.tile([1, 128], FP)                 # lhsT for bcast outer product
    nc.sync.dma_start(ones_col[:], ins["ones_col"][:])
    ones_k = persist.tile([128, 1], FP)                   # lhsT for partition sums
    nc.sync.dma_start(ones_k[:], ins["ones_k"][:])
    e_sb = persist.tile([128, KV, c.D], FP)               # embedding as lhsT
    nc.sync.dma_start(e_sb[:], ins["E_lhsT"][:])
    et_sb = persist.tile([128, KD, c.V], FP)              # (E*gamma_f).T as rhs
    nc.sync.dma_start(et_sb[:], ins["ET_rhs"][:])
    bv_sb = persist.tile([1, c.V], FP)                    # E @ beta_f logits bias row
    nc.sync.dma_start(bv_sb[:], ins["bv_row"][:])
    eps_sb = persist.tile([1, 1], FP)
    nc.vector.memset(eps_sb[:], float(c.EPS))
    id_sb = persist.tile([128, 128], SDT)                 # identity for psum-accumulate adds
    nc.sync.dma_start(id_sb[:], ins["ident"][:])

    # ---- per-layer weight tiles (reloaded every layer) ----
    uzr_sb = wpool.tile([128, KD, 2 * c.DI], SDT)
    uh_sb = wpool.tile([128, KD, c.DI], SDT)
    wzr_sb = wpool.tile([128, KD, 2 * c.DI], FP)
    wh_sb = wpool.tile([128, KD, c.DI], FP)
    wo_sb = wpool.tile([128, KD, c.D], FP)
    bzr_sb = wpool.tile([128, MZR], FP)
    bh_sb = wpool.tile([128, MH], FP)

    def layer_norm_chunk(col0, n, src_tile, dst_tile, src_dyn):
        """dst = (src - mean) * rsqrt(var + eps) per column, stats over partitions.

        src is [128, KD, *]; col0 is a python int or ScalarValue column offset.
        src_dyn: slice helper for src's col dim."""
        # partition sums via ones-matmul: mean and mean-square rows [1, n]
        mean_ps = ps_bc.tile([1, n], FP, tag="bc")
        for k in range(KD):
            nc.tensor.matmul(mean_ps[:], ones_k[:], src_tile[:, k, src_dyn(col0, n)],
                             start=(k == 0), stop=(k == KD - 1))
        for k in range(KD):
            nc.scalar.activation(sq_sb[:, k, 0:n], src_tile[:, k, src_dyn(col0, n)], AF.Square)
        sq_ps = ps_bc.tile([1, n], FP, tag="bc")
        for k in range(KD):
            nc.tensor.matmul(sq_ps[:], ones_k[:], sq_sb[:, k, 0:n],
                             start=(k == 0), stop=(k == KD - 1))
        mean_row = sb.tile([1, n], FP, tag="row", bufs=8)
        nc.vector.tensor_scalar(mean_row[:], mean_ps[:], 1.0 / c.D, None, ALU.mult)
        msq_row = sb.tile([1, n], FP, tag="row", bufs=8)
        nc.vector.tensor_scalar(msq_row[:], sq_ps[:], 1.0 / c.D, None, ALU.mult)
        var_row = sb.tile([1, n], FP, tag="row", bufs=8)
        nc.vector.tensor_tensor(var_row[:], mean_row[:], mean_row[:], ALU.mult)
        nc.vector.tensor_tensor(var_row[:], msq_row[:], var_row[:], ALU.subtract)
        std_row = sb.tile([1, n], FP, tag="row", bufs=8)
        nc.scalar.activation(std_row[:], var_row[:], AF.Sqrt, bias=eps_sb[:])
        rstd_row = sb.tile([1, n], FP, tag="row", bufs=8)
        nc.vector.reciprocal(rstd_row[:], std_row[:])
        mr_row = sb.tile([1, n], FP, tag="row", bufs=8)
        nc.vector.tensor_tensor(mr_row[:], mean_row[:], rstd_row[:], ALU.mult)
        # broadcast rows to all 128 partitions via outer product with ones
        rb_ps = ps_bc.tile([128, n], FP, tag="bc")
        nc.tensor.matmul(rb_ps[:], ones_col[:], rstd_row[:], start=True, stop=True)
        mrb_ps = ps_bc.tile([128, n], FP, tag="bc")
        nc.tensor.matmul(mrb_ps[:], ones_col[:], mr_row[:], start=True, stop=True)
        for k in range(KD):
            nc.vector.tensor_tensor(dst_tile[:, k, 0:n], src_tile[:, k, src_dyn(col0, n)],
                                    rb_ps[:], ALU.mult)
            nc.vector.tensor_tensor(dst_tile[:, k, 0:n], dst_tile[:, k, 0:n],
                                    mrb_ps[:], ALU.subtract)

    def dyn(col0, n):
        if isinstance(col0, int):
            return slice(col0, col0 + n)
        return bass.ds(col0, n)

    # ================= embedding: one-hot matmul =================
    ECW = min(512, c.NTOK)
    for ec in range(c.NTOK // ECW):
        x_row = sb.tile([1, ECW], FP, tag="xrow")
        nc.sync.dma_start(x_row[:], ins["x_tb"][:, ec * ECW:(ec + 1) * ECW])
        xb_ps = ps_bc.tile([128, ECW], FP, tag="bc")
        nc.tensor.matmul(xb_ps[:], ones_col[:], x_row[:], start=True, stop=True)
        ohs = []
        for vc in range(KV):
            oh = sb.tile([128, ECW], FP, tag=f"oh{vc}")
            nc.vector.tensor_scalar(oh[:], xb_ps[:], iota2[:, vc:vc + 1], None, ALU.is_equal)
            ohs.append(oh)
        for dm in range(KD):
            px = ps_proj.tile([128, ECW], FP, tag="px")
            for vc in range(KV):
                nc.tensor.matmul(px[:], e_sb[:, vc, dm * 128:(dm + 1) * 128], ohs[vc][:],
                                 start=(vc == 0), stop=(vc == KV - 1))
            nc.vector.tensor_copy(h_sb[:, dm, ec * ECW:(ec + 1) * ECW], px[:])

    # ================= layers =================
    for layer in range(c.DEPTH):
        nc.sync.dma_start(uzr_sb[:], ins["UzrT_all"][layer][:])
        nc.sync.dma_start(uh_sb[:], ins["UhT_all"][layer][:])
        nc.sync.dma_start(wzr_sb[:], ins["WzrT_all"][layer][:])
        nc.sync.dma_start(wh_sb[:], ins["WhT_all"][layer][:])
        nc.sync.dma_start(wo_sb[:], ins["WoT_all"][layer][:])
        nc.sync.dma_start(bzr_sb[:], ins["bzr_all"][layer][:])
        nc.sync.dma_start(bh_sb[:], ins["bh_all"][layer][:])
        nc.vector.memset(hsT[:, :, 0:BL], 0.0)

        with tc.For_i(0, c.NCHUNK) as cc:
            ccol = cc * CC
            # ---- A: LN + input projections for this chunk ----
            layer_norm_chunk(ccol, CC, h_sb, hn_sb, dyn)
            for m in range(MZR):
                px = ps_proj.tile([128, CC], FP, tag="px")
                for k in range(KD):
                    nc.tensor.matmul(px[:], wzr_sb[:, k, m * 128:(m + 1) * 128],
                                     hn_sb[:, k, 0:CC], start=(k == 0), stop=(k == KD - 1))
                nc.scalar.activation(xzrT[:, m, 0:CC], px[:], AF.Identity,
                                     bias=bzr_sb[:, m:m + 1])
            for m in range(MH):
                px = ps_proj.tile([128, CC], FP, tag="px")
                for k in range(KD):
                    nc.tensor.matmul(px[:], wh_sb[:, k, m * 128:(m + 1) * 128],
                                     hn_sb[:, k, 0:CC], start=(k == 0), stop=(k == KD - 1))
                nc.scalar.activation(xhT[:, m, 0:CC], px[:], AF.Identity,
                                     bias=bh_sb[:, m:m + 1])

            # ---- B: the GRU scan over S steps ----
            # Per step: r gates first so the U_h matmul chain overlaps the
            # z-gate matmuls; matmul inputs optionally cast to bf16 (FWL 2x).
            MR = MZR // 2
            with tc.For_i(0, S, U, hint_engines=(mybir.EngineType.PE,)) as it:
                hb_prev = None
                for u in range(U):
                    cin = bass.ds((it + u) * BL, BL)
                    cout = bass.ds((it + u + 1) * BL, BL)
                    if c.bf16_scan:
                        # bf16 copy of the state for matmul rhs: produced by the
                        # PREVIOUS step's blend (hb_prev); first step of the body
                        # trip casts from hsT.
                        if hb_prev is None:
                            hb = sb.tile([128, KD, BL], BF, tag="hb", bufs=3)
                            nc.vector.tensor_copy(hb[:], hsT[:, 0:KD, cin])
                        else:
                            hb = hb_prev
                        rhs_h = [hb[:, k, :] for k in range(KD)]
                    else:
                        rhs_h = [hsT[:, k, cin] for k in range(KD)]
                    zrr_ps = ps_scan.tile([128, MH * BL], FP, tag="zrr", bufs=1)
                    for m in range(MR, MZR):
                        for k in range(KD):
                            nc.tensor.matmul(zrr_ps[:, (m - MR) * BL:(m - MR + 1) * BL],
                                             uzr_sb[:, k, m * 128:(m + 1) * 128],
                                             rhs_h[k],
                                             start=(k == 0), stop=(k == KD - 1))
                    zs_r = sb.tile([128, MH * BL], FP, tag="zs_r", bufs=3)
                    nc.vector.tensor_tensor(zs_r[:], zrr_ps[:], xzrT[:, MR:MZR, cin], ALU.add)
                    za_r = sb.tile([128, MH * BL], FP, tag="za_r", bufs=3)
                    nc.scalar.activation(za_r[:], zs_r[:], AF.Sigmoid)
                    rh = sb.tile([128, KD, BL], SDT, tag="rh", bufs=3)
                    nc.vector.tensor_tensor(rh[:], za_r[:], hsT[:, 0:KD, cin], ALU.mult)
                    # z gates fill the PE while the r chain runs on DVE/ACT
                    zrz_ps = ps_scan.tile([128, MH * BL], FP, tag="zrz", bufs=1)
                    for m in range(0, MR):
                        for k in range(KD):
                            nc.tensor.matmul(zrz_ps[:, m * BL:(m + 1) * BL],
                                             uzr_sb[:, k, m * 128:(m + 1) * 128],
                                             rhs_h[k],
                                             start=(k == 0), stop=(k == KD - 1))
                    zs_z = sb.tile([128, MH * BL], FP, tag="zs_z", bufs=3)
                    nc.vector.tensor_tensor(zs_z[:], zrz_ps[:], xzrT[:, 0:MR, cin], ALU.add)
                    za_z = sb.tile([128, MH * BL], FP, tag="za_z", bufs=3)
                    nc.scalar.activation(za_z[:], zs_z[:], AF.Sigmoid)
                    omz = sb.tile([128, MH * BL], FP, tag="omz", bufs=3)
                    nc.vector.tensor_scalar(omz[:], za_z[:], -1.0, 1.0, ALU.mult, ALU.add)
                    c1 = sb.tile([128, MH * BL], FP, tag="c1", bufs=3)
                    nc.vector.tensor_tensor(c1[:], omz[:], hsT[:, 0:KD, cin], ALU.mult)
                    hp_ps = ps_scan.tile([128, MH * BL], FP, tag="hp", bufs=2)
                    for m in range(MH):
                        for k in range(KD):
                            nc.tensor.matmul(hp_ps[:, m * BL:(m + 1) * BL],
                                             uh_sb[:, k, m * 128:(m + 1) * 128],
                                             rh[:, k, :],
                                             start=(k == 0), stop=(k == KD - 1))
                    hs_t = sb.tile([128, MH * BL], FP, tag="hs_t", bufs=3)
                    nc.vector.tensor_tensor(hs_t[:], hp_ps[:], xhT[:, 0:MH, cin], ALU.add)
                    hc = sb.tile([128, MH * BL], FP, tag="hc", bufs=3)
                    nc.scalar.activation(hc[:], hs_t[:], AF.Tanh)
                    zd = sb.tile([128, MH * BL], FP, tag="zd", bufs=3)
                    nc.vector.tensor_tensor(zd[:], za_z[:], hc[:], ALU.mult)
                    if c.bf16_scan:
                        hb_next = sb.tile([128, KD, BL], BF, tag="hb", bufs=3)
                        nc.vector.tensor_tensor(hb_next[:], c1[:], zd[:], ALU.add)
                        hb_prev = hb_next
                    nc.vector.tensor_tensor(hsT[:, 0:KD, cout], c1[:],
                                            zd[:], ALU.add)

            # ---- C: output projection + residual ----
            for dm in range(KD):
                po = ps_proj.tile([128, CC], FP, tag="px")
                for k in range(KD):
                    nc.tensor.matmul(po[:], wo_sb[:, k, dm * 128:(dm + 1) * 128],
                                     hsT[:, k, BL:(S + 1) * BL],
                                     start=(k == 0), stop=(k == KD - 1))
                nc.vector.tensor_tensor(h_sb[:, dm, dyn(ccol, CC)],
                                        h_sb[:, dm, dyn(ccol, CC)], po[:], ALU.add)
            # carry the state to column 0 for the next chunk
            nc.vector.tensor_copy(hsT[:, :, 0:BL], hsT[:, :, S * BL:(S + 1) * BL])

    # ================= final LN + logits =================
    W = min(128, CC)
    for ec in range(c.NTOK // CC):
        layer_norm_chunk(ec * CC, CC, h_sb, hn_sb, dyn)
        for t4 in range(CC // W):
            pl = ps_proj.tile([128, c.V], FP, tag="px")
            for k in range(KD):
                nc.tensor.matmul(pl[:W], hn_sb[:, k, t4 * W:(t4 + 1) * W],
                                 et_sb[:, k, :], start=(k == 0), stop=False)
            nc.tensor.matmul(pl[:W], ones_col[:, 0:W], bv_sb[:], start=False, stop=True)
            out_sb = sb.tile([128, c.V], FP, tag="osb")
            nc.vector.tensor_copy(out_sb[:W], pl[:W])
            r0 = ec * CC + t4 * W
            nc.sync.dma_start(logits[r0:r0 + W, :], out_sb[:W])


# ======================= host side =======================

def _pack_lhsT(m, kchunks, dtype=np.float32):
    # m: [K, J] with K = kchunks*128  ->  [128, kchunks, J]
    K, J = m.shape
    assert K == kchunks * 128
    return np.ascontiguousarray(m.reshape(kchunks, 128, J).transpose(1, 0, 2),
                                dtype=dtype)


def prep_inputs(inputs, cfg: Cfg):
    c = cfg
    f8 = np.float64
    x = np.asarray(inputs["x"])
    emb = np.asarray(inputs["embedding"], f8)
    ln_g = np.asarray(inputs["ln_gamma"], f8)
    ln_b = np.asarray(inputs["ln_beta"], f8)
    Win = np.asarray(inputs["Win"], f8)
    W_zr = np.asarray(inputs["W_zr"], f8)
    U_zr = np.asarray(inputs["U_zr"], f8)
    W_h = np.asarray(inputs["W_h"], f8)
    U_h = np.asarray(inputs["U_h"], f8)
    b_zr = np.asarray(inputs["b_zr"], f8)
    b_h = np.asarray(inputs["b_h"], f8)
    Wout = np.asarray(inputs["Wout"], f8)
    ng = np.asarray(inputs["norm_gamma"], f8)
    nb = np.asarray(inputs["norm_beta"], f8)

    shared = {}
    L = c.DEPTH
    if c.bf16_scan:
        import ml_dtypes
        sdt = ml_dtypes.bfloat16
    else:
        sdt = np.float32
    shared["UzrT_all"] = np.stack([_pack_lhsT(U_zr[l].T, c.KD, sdt) for l in range(L)])
    shared["UhT_all"] = np.stack([_pack_lhsT(U_h[l].T, c.KD, sdt) for l in range(L)])
    wzr_l, wh_l, bzr_l, bh_l, wo_l = [], [], [], [], []
    for l in range(L):
        Wzr_eff = W_zr[l] @ Win[l]                     # [2DI, D]
        bzr_eff = Wzr_eff @ ln_b[l] + b_zr[l]
        Wzr_eff = Wzr_eff * ln_g[l][None, :]
        Wh_eff = W_h[l] @ Win[l]
        bh_eff = Wh_eff @ ln_b[l] + b_h[l]
        Wh_eff = Wh_eff * ln_g[l][None, :]
        wzr_l.append(_pack_lhsT(Wzr_eff.T, c.KD))
        wh_l.append(_pack_lhsT(Wh_eff.T, c.KD))
        bzr_l.append(np.ascontiguousarray(
            bzr_eff.reshape(c.MZR, 128).T, dtype=np.float32))
        bh_l.append(np.ascontiguousarray(
            bh_eff.reshape(c.MH, 128).T, dtype=np.float32))
        wo_l.append(_pack_lhsT(Wout[l].T, c.KD))
    shared["WzrT_all"] = np.stack(wzr_l)
    shared["WhT_all"] = np.stack(wh_l)
    shared["bzr_all"] = np.stack(bzr_l)
    shared["bh_all"] = np.stack(bh_l)
    shared["WoT_all"] = np.stack(wo_l)
    shared["E_lhsT"] = np.ascontiguousarray(
        emb.reshape(c.KV, 128, c.D).transpose(1, 0, 2), dtype=np.float32)
    shared["ET_rhs"] = _pack_lhsT((emb * ng[None, :]).T, c.KD)
    shared["bv_row"] = np.ascontiguousarray((emb @ nb)[None, :], dtype=np.float32)
    shared["iota2"] = np.ascontiguousarray(
        (np.arange(128)[:, None] + 128 * np.arange(c.KV)[None, :]), dtype=np.float32)
    shared["ident"] = np.eye(128, dtype=sdt)
    shared["ones_col"] = np.ones((1, 128), np.float32)
    shared["ones_k"] = np.ones((128, 1), np.float32)

    in_maps = []
    for core in range(c.n_cores):
        xc = x[core * c.BL:(core + 1) * c.BL, :]        # [BL, T]
        x_tb = np.ascontiguousarray(xc.T.reshape(1, -1), dtype=np.float32)
        m = dict(shared)
        m["x_tb"] = x_tb
        in_maps.append(m)
    return in_maps, shared


def declare_tensors(nc, cfg: Cfg, shared):
    c = cfg
    ins = {}
    ins["x_tb"] = nc.dram_tensor("x_tb", [1, c.NTOK], FP, kind="ExternalInput").ap()
    for name, arr in shared.items():
        if name == "x_tb":
            continue
        dt = mybir.dt.from_np(arr.dtype)
        ins[name] = nc.dram_tensor(name, list(arr.shape), dt, kind="ExternalInput").ap()
    outs = {}
    outs["logits"] = nc.dram_tensor("logits", [c.NTOK, c.V], FP,
                                    kind="ExternalOutput").ap()
    return outs, ins


_CACHE = {}


def build_program(cfg: Cfg, shared, enable_asserts=False):
    key = (cfg.DEPTH, cfg.T, cfg.S, cfg.U, cfg.n_cores, cfg.bf16_scan)
    if key in _CACHE:
        return _CACHE[key]
    nc = bacc.Bacc("TRN2", target_bir_lowering=False, debug=False,
                   enable_asserts=enable_asserts, num_devices=cfg.n_cores)
    outs, ins = declare_tensors(nc, cfg, shared)
    with tile.TileContext(nc) as tc:
        with ExitStack() as ctx:
            build_kernel(ctx, tc, outs, ins, cfg)
    nc.compile()
    _CACHE[key] = nc
    return nc


def kernel(**inputs) -> np.ndarray:
    cfg = Cfg()
    in_maps, shared = prep_inputs(inputs, cfg)
    nc = build_program(cfg, shared)
    res = run_bass_kernel_spmd(nc, in_maps, core_ids=list(range(cfg.n_cores)))
    outs = []
    for core in range(cfg.n_cores):
        lg = res.results[core]["logits"]               # [NTOK, V], token = t*BL+b
        lg = lg.reshape(cfg.T, cfg.BL, cfg.V).transpose(1, 0, 2)
        outs.append(lg)
    return np.ascontiguousarray(np.concatenate(outs, axis=0), dtype=np.float32)


if __name__ == "__main__":
    # smoke test with random data
    rng = np.random.default_rng(0)
    cfg = Cfg()
    ins = dict(
        x=rng.integers(0, 256, size=(16, 2048)),
        embedding=rng.normal(size=(256, 512)).astype(np.float32) * 0.02,
        ln_gamma=np.ones((12, 512), np.float32),
        ln_beta=np.zeros((12, 512), np.float32),
        Win=rng.normal(size=(12, 512, 512)).astype(np.float32) * 0.02,
        W_zr=rng.normal(size=(12, 1024, 512)).astype(np.float32) * 0.02,
        U_zr=rng.normal(size=(12, 1024, 512)).astype(np.float32) * 0.04,
        W_h=rng.normal(size=(12, 512, 512)).astype(np.float32) * 0.02,
        U_h=rng.normal(size=(12, 512, 512)).astype(np.float32) * 0.04,
        b_zr=np.zeros((12, 1024), np.float32),
        b_h=np.zeros((12, 512), np.float32),
        Wout=rng.normal(size=(12, 512, 512)).astype(np.float32) * 0.02,
        norm_gamma=np.ones((512,), np.float32),
        norm_beta=np.zeros((512,), np.float32),
    )
    out = kernel(**ins)
    print(out.shape, out.dtype, np.abs(out).max())



# revision 7
# speedup vs baseline: 1.2920x; 1.2920x over previous
"""Trainium2 Bass kernel for a 12-layer GRU LM (nn_CudaGRULM).

Model: h = emb[x]; 12x { residual + Wout @ GRU(Win @ LN(h)) }; LN; logits = h @ emb.T
Shapes: V=256, D=512, DEPTH=12, DI=512, B=16, T=2048.

Strategy:
 - Data-parallel over batch: 8 cores x B_local=2. No collectives; each core
   computes its batch shard end-to-end and returns its logits shard.
 - Band-pipelined wavefront: layers are processed in bands of W adjacent
   layers; within a band, a chunk loop runs W independent scan chains
   interleaved (chain j handles layer W*b+j on chunk k-j), so the W
   recurrences hide each other's per-step latency and keep the PE fed.
 - Everything on device lives in "T-layout": feature dim on partitions
   (4 chunks of 128), tokens along the free dim (col = t*B_local + b).
 - Host-side exact algebra: LN gamma/beta folded into fused input-projection
   weights (W_zr@Win etc. in float64); embedding gather via one-hot matmul.
 - Scan: recurrent weights stationary on the PE (LDWEIGHTS-bound); state
   kept fp32 in a ring, with a parallel bf16 ring feeding matmul rhs and
   the output projection.
"""

from contextlib import ExitStack

import numpy as np

import concourse.bass as bass
import concourse.bacc as bacc
import concourse.tile as tile
from concourse import mybir
from concourse.bass_utils import run_bass_kernel_spmd

FP = mybir.dt.float32
BF = mybir.dt.bfloat16
AF = mybir.ActivationFunctionType
ALU = mybir.AluOpType


class Cfg:
    def __init__(self, V=256, D=512, DEPTH=12, DI=512, B=16, T=2048,
                 n_cores=8, S=128, U=16, W=2, EPS=1e-5):
        self.V, self.D, self.DEPTH, self.DI, self.B, self.T = V, D, DEPTH, DI, B, T
        self.n_cores = n_cores
        self.BL = B // n_cores          # local batch per core
        self.NTOK = T * self.BL         # tokens per core
        self.S = S                      # scan steps per chunk
        self.U = U                      # scan unroll inside For_i
        self.W = W                      # band width (interleaved chains)
        self.NCHUNK = T // S
        self.NITER = self.NCHUNK + W - 1
        self.NBAND = DEPTH // W
        self.CC = S * self.BL           # chunk cols
        self.PAD = W - 1                # pad chunks on each side of h_sb
        self.HCOL = (self.NCHUNK + 2 * self.PAD) * self.CC
        self.EPS = EPS
        self.KD = D // 128              # 4  (feature chunks)
        self.KV = V // 128              # 2  (vocab chunks)
        self.MZR = 2 * DI // 128        # 8  (zr gate chunks)
        self.MH = DI // 128             # 4  (h gate chunks)
        assert D == DI and DEPTH % W == 0
        assert self.S % self.U == 0 and T % S == 0 and B % n_cores == 0


def build_kernel(ctx: ExitStack, tc: "tile.TileContext", outs, ins, cfg: Cfg):
    nc = tc.nc
    c = cfg
    KD, KV, MZR, MH, BL, CC, S, U, W = (c.KD, c.KV, c.MZR, c.MH, c.BL, c.CC,
                                        c.S, c.U, c.W)

    logits = outs["logits"]

    persist = ctx.enter_context(tc.tile_pool(name="persist", bufs=1))
    wpool = ctx.enter_context(tc.tile_pool(name="wpool", bufs=1))
    sb = ctx.enter_context(tc.tile_pool(name="sb", bufs=2))
    ps_proj = ctx.enter_context(tc.tile_pool(name="ps_proj", bufs=1, space="PSUM"))
    ps_bc = ctx.enter_context(tc.tile_pool(name="ps_bc", bufs=2, space="PSUM"))
    ps_sc = [ctx.enter_context(tc.tile_pool(name=f"ps_sc{j}", bufs=1, space="PSUM"))
             for j in range(W)]

    # ---- persistent state ----
    h_sb = persist.tile([128, KD, c.HCOL], FP)            # residual stream (padded)
    hsT = [persist.tile([128, KD, (S + 1) * BL], FP, name=f"hsT{j}") for j in range(W)]
    hsB = [persist.tile([128, KD, (S + 1) * BL], BF, name=f"hsB{j}") for j in range(W)]
    xzrT = [persist.tile([128, MZR, CC], FP, name=f"xzrT{j}") for j in range(W)]
    xhT = [persist.tile([128, MH, CC], FP, name=f"xhT{j}") for j in range(W)]
    hn_sb = persist.tile([128, KD, CC], BF)               # normalized chunk (shared)
    sq_sb = persist.tile([128, KD, CC], FP)               # squared chunk (LN stats)

    # ---- constants ----
    iota2 = persist.tile([128, KV], FP)
    nc.sync.dma_start(iota2[:], ins["iota2"][:])
    ones_col = persist.tile([1, 128], FP)                 # lhsT for bcast outer product
    nc.sync.dma_start(ones_col[:], ins["ones_col"][:])
    ones_k = persist.tile([128, 1], FP)                   # lhsT for partition sums
    nc.sync.dma_start(ones_k[:], ins["ones_k"][:])
    e_sb = persist.tile([128, KV, c.D], BF)               # embedding as lhsT
    nc.sync.dma_start(e_sb[:], ins["E_lhsT"][:])
    et_sb = persist.tile([128, KD, c.V], FP)              # (E*gamma_f).T as rhs
    nc.sync.dma_start(et_sb[:], ins["ET_rhs"][:])
    bv_sb = persist.tile([1, c.V], FP)                    # E @ beta_f logits bias row
    nc.sync.dma_start(bv_sb[:], ins["bv_row"][:])
    gam_sb = persist.tile([128, W, c.NITER], FP)          # per-chain carry masks
    nc.sync.dma_start(gam_sb[:], ins["gam"][:])
    eps_sb = persist.tile([1, 1], FP)
    nc.vector.memset(eps_sb[:], float(c.EPS))

    # zero the pad chunks of h_sb and the scan rings (NaN hygiene: garbage
    # edge chunks must stay finite so masked carries can zero them)
    nc.vector.memset(h_sb[:, :, 0:c.PAD * CC], 0.0)
    p0 = (c.PAD + c.NCHUNK) * CC
    nc.vector.memset(h_sb[:, :, p0:p0 + c.PAD * CC], 0.0)
    for j in range(W):
        nc.vector.memset(hsT[j][:], 0.0)
        nc.vector.memset(hsB[j][:], 0.0)

    # ---- per-band weight tiles (reloaded every band) ----
    uzr_w = [wpool.tile([128, KD, 2 * c.DI], BF, name=f"uzr_w{j}") for j in range(W)]
    uh_w = [wpool.tile([128, KD, c.DI], BF, name=f"uh_w{j}") for j in range(W)]
    wzr_w = [wpool.tile([128, KD, 2 * c.DI], BF, name=f"wzr_w{j}") for j in range(W)]
    wh_w = [wpool.tile([128, KD, c.DI], BF, name=f"wh_w{j}") for j in range(W)]
    wo_w = [wpool.tile([128, KD, c.D], BF, name=f"wo_w{j}") for j in range(W)]
    bzr_w = [wpool.tile([128, MZR], FP, name=f"bzr_w{j}") for j in range(W)]
    bh_w = [wpool.tile([128, MH], FP, name=f"bh_w{j}") for j in range(W)]

    def layer_norm_chunk(col0, n, src_tile, dst_tile):
        """dst = (src - mean) * rsqrt(var + eps) per column (dst may be bf16)."""
        mean_ps = ps_bc.tile([1, n], FP, tag="bc")
        for k in range(KD):
            nc.tensor.matmul(mean_ps[:], ones_k[:], src_tile[:, k, dyn(col0, n)],
                             start=(k == 0), stop=(k == KD - 1))
        for k in range(KD):
            nc.scalar.activation(sq_sb[:, k, 0:n], src_tile[:, k, dyn(col0, n)],
                                 AF.Square)
        sq_ps = ps_bc.tile([1, n], FP, tag="bc")
        for k in range(KD):
            nc.tensor.matmul(sq_ps[:], ones_k[:], sq_sb[:, k, 0:n],
                             start=(k == 0), stop=(k == KD - 1))
        mean_row = sb.tile([1, n], FP, tag="row", bufs=4)
        nc.vector.tensor_scalar(mean_row[:], mean_ps[:], 1.0 / c.D, None, ALU.mult)
        msq_row = sb.tile([1, n], FP, tag="row", bufs=4)
        nc.vector.tensor_scalar(msq_row[:], sq_ps[:], 1.0 / c.D, None, ALU.mult)
        var_row = sb.tile([1, n], FP, tag="row", bufs=4)
        nc.vector.tensor_tensor(var_row[:], mean_row[:], mean_row[:], ALU.mult)
        nc.vector.tensor_tensor(var_row[:], msq_row[:], var_row[:], ALU.subtract)
        std_row = sb.tile([1, n], FP, tag="row", bufs=4)
        nc.scalar.activation(std_row[:], var_row[:], AF.Sqrt, bias=eps_sb[:])
        rstd_row = sb.tile([1, n], FP, tag="row", bufs=4)
        nc.vector.reciprocal(rstd_row[:], std_row[:])
        mr_row = sb.tile([1, n], FP, tag="row", bufs=4)
        nc.vector.tensor_tensor(mr_row[:], mean_row[:], rstd_row[:], ALU.mult)
        rb_ps = ps_bc.tile([128, n], FP, tag="bcb")
        nc.tensor.matmul(rb_ps[:], ones_col[:], rstd_row[:], start=True, stop=True)
        mrb_ps = ps_bc.tile([128, n], FP, tag="bcb")
        nc.tensor.matmul(mrb_ps[:], ones_col[:], mr_row[:], start=True, stop=True)
        for k in range(KD):
            tmp = sb.tile([128, n], FP, tag="lnt", bufs=2)
            nc.vector.tensor_tensor(tmp[:], src_tile[:, k, dyn(col0, n)],
                                    rb_ps[:], ALU.mult)
            nc.vector.tensor_tensor(dst_tile[:, k, 0:n], tmp[:],
                                    mrb_ps[:], ALU.subtract)

    def dyn(col0, n):
        if isinstance(col0, int):
            return slice(col0, col0 + n)
        return bass.ds(col0, n)

    # ================= embedding: one-hot matmul =================
    ECW = min(512, c.NTOK)
    for ec in range(c.NTOK // ECW):
        x_row = sb.tile([1, ECW], FP, tag="xrow")
        nc.sync.dma_start(x_row[:], ins["x_tb"][:, ec * ECW:(ec + 1) * ECW])
        xb_ps = ps_bc.tile([128, ECW], FP, tag="bcb")
        nc.tensor.matmul(xb_ps[:], ones_col[:], x_row[:], start=True, stop=True)
        ohs = []
        for vc in range(KV):
            oh = sb.tile([128, ECW], BF, tag=f"oh{vc}")
            nc.vector.tensor_scalar(oh[:], xb_ps[:], iota2[:, vc:vc + 1], None,
                                    ALU.is_equal)
            ohs.append(oh)
        for dm in range(KD):
            px = ps_proj.tile([128, ECW], FP, tag="px", bufs=2)
            for vc in range(KV):
                nc.tensor.matmul(px[:], e_sb[:, vc, dm * 128:(dm + 1) * 128],
                                 ohs[vc][:], start=(vc == 0), stop=(vc == KV - 1))
            dst0 = c.PAD * CC + ec * ECW
            nc.vector.tensor_copy(h_sb[:, dm, dst0:dst0 + ECW], px[:])

    # ================= bands =================
    for b in range(c.NBAND):
        for j in range(W):
            lay = b * W + j
            nc.sync.dma_start(uzr_w[j][:], ins["UzrT_all"][lay][:])
            nc.sync.dma_start(uh_w[j][:], ins["UhT_all"][lay][:])
            nc.sync.dma_start(wzr_w[j][:], ins["WzrT_all"][lay][:])
            nc.sync.dma_start(wh_w[j][:], ins["WhT_all"][lay][:])
            nc.sync.dma_start(wo_w[j][:], ins["WoT_all"][lay][:])
            nc.sync.dma_start(bzr_w[j][:], ins["bzr_all"][lay][:])
            nc.sync.dma_start(bh_w[j][:], ins["bh_all"][lay][:])

        with tc.For_i(0, c.NITER) as it:
            # ---- A: LN + input projections, one per chain ----
            for j in range(W):
                # chunk index it-j at padded offset (it-j+PAD)*CC
                col0 = (it + (c.PAD - j)) * CC
                layer_norm_chunk(col0, CC, h_sb, hn_sb)
                for m in range(MZR):
                    px = ps_proj.tile([128, CC], FP, tag="px", bufs=2)
                    for k in range(KD):
                        nc.tensor.matmul(px[:], wzr_w[j][:, k, m * 128:(m + 1) * 128],
                                         hn_sb[:, k, 0:CC],
                                         start=(k == 0), stop=(k == KD - 1))
                    nc.scalar.activation(xzrT[j][:, m, 0:CC], px[:], AF.Identity,
                                         bias=bzr_w[j][:, m:m + 1])
                for m in range(MH):
                    px = ps_proj.tile([128, CC], FP, tag="px", bufs=2)
                    for k in range(KD):
                        nc.tensor.matmul(px[:], wh_w[j][:, k, m * 128:(m + 1) * 128],
                                         hn_sb[:, k, 0:CC],
                                         start=(k == 0), stop=(k == KD - 1))
                    nc.scalar.activation(xhT[j][:, m, 0:CC], px[:], AF.Identity,
                                         bias=bh_w[j][:, m:m + 1])

            # ---- B: W interleaved GRU scans over S steps ----
            MR = MZR // 2
            with tc.For_i(0, S, U, hint_engines=(mybir.EngineType.PE,)) as st:
                for u in range(U):
                    cin = bass.ds((st + u) * BL, BL)
                    cout = bass.ds((st + u + 1) * BL, BL)
                    for j in range(W):
                        rhs_h = [hsB[j][:, k, cin] for k in range(KD)]
                        scps = ps_sc[j].tile([128, 3 * MH * BL], FP, tag="s",
                                             bufs=1)
                        zrr_ps = scps[:, 0:MH * BL]
                        zrz_ps = scps[:, MH * BL:2 * MH * BL]
                        hp_ps = scps[:, 2 * MH * BL:3 * MH * BL]
                        for m in range(MR, MZR):
                            for k in range(KD):
                                nc.tensor.matmul(
                                    zrr_ps[:, (m - MR) * BL:(m - MR + 1) * BL],
                                    uzr_w[j][:, k, m * 128:(m + 1) * 128], rhs_h[k],
                                    start=(k == 0), stop=(k == KD - 1))
                        zs_r = sb.tile([128, MH * BL], FP, tag=f"zs_r{j}", bufs=3)
                        nc.vector.tensor_tensor(zs_r[:], zrr_ps[:],
                                                xzrT[j][:, MR:MZR, cin], ALU.add)
                        za_r = sb.tile([128, MH * BL], FP, tag=f"za_r{j}", bufs=3)
                        nc.scalar.activation(za_r[:], zs_r[:], AF.Sigmoid)
                        rh = sb.tile([128, KD, BL], BF, tag=f"rh{j}", bufs=3)
                        nc.vector.tensor_tensor(rh[:], za_r[:], hsT[j][:, 0:KD, cin],
                                                ALU.mult)
                        for m in range(0, MR):
                            for k in range(KD):
                                nc.tensor.matmul(
                                    zrz_ps[:, m * BL:(m + 1) * BL],
                                    uzr_w[j][:, k, m * 128:(m + 1) * 128], rhs_h[k],
                                    start=(k == 0), stop=(k == KD - 1))
                        zs_z = sb.tile([128, MH * BL], FP, tag=f"zs_z{j}", bufs=3)
                        nc.vector.tensor_tensor(zs_z[:], zrz_ps[:],
                                                xzrT[j][:, 0:MR, cin], ALU.add)
                        za_z = sb.tile([128, MH * BL], FP, tag=f"za_z{j}", bufs=3)
                        nc.scalar.activation(za_z[:], zs_z[:], AF.Sigmoid)
                        omz = sb.tile([128, MH * BL], FP, tag=f"omz{j}", bufs=3)
                        nc.vector.tensor_scalar(omz[:], za_z[:], -1.0, 1.0,
                                                ALU.mult, ALU.add)
                        c1 = sb.tile([128, MH * BL], FP, tag=f"c1{j}", bufs=3)
                        nc.vector.tensor_tensor(c1[:], omz[:], hsT[j][:, 0:KD, cin],
                                                ALU.mult)
                        for m in range(MH):
                            for k in range(KD):
                                nc.tensor.matmul(
                                    hp_ps[:, m * BL:(m + 1) * BL],
                                    uh_w[j][:, k, m * 128:(m + 1) * 128],
                                    rh[:, k, :], start=(k == 0), stop=(k == KD - 1))
                        hs_t = sb.tile([128, MH * BL], FP, tag=f"hs_t{j}", bufs=3)
                        nc.vector.tensor_tensor(hs_t[:], hp_ps[:],
                                                xhT[j][:, 0:MH, cin], ALU.add)
                        hc = sb.tile([128, MH * BL], FP, tag=f"hc{j}", bufs=3)
                        nc.scalar.activation(hc[:], hs_t[:], AF.Tanh)
                        zd = sb.tile([128, MH * BL], FP, tag=f"zd{j}", bufs=3)
                        nc.vector.tensor_tensor(zd[:], za_z[:], hc[:], ALU.mult)
                        nc.vector.tensor_tensor(hsT[j][:, 0:KD, cout], c1[:],
                                                zd[:], ALU.add)
                        nc.vector.tensor_tensor(hsB[j][:, 0:KD, cout], c1[:],
                                                zd[:], ALU.add)

            # ---- C: output projection + residual, one per chain ----
            for j in range(W):
                col0 = (it + (c.PAD - j)) * CC
                for dm in range(KD):
                    po = ps_proj.tile([128, CC], FP, tag="px", bufs=2)
                    for k in range(KD):
                        nc.tensor.matmul(po[:], wo_w[j][:, k, dm * 128:(dm + 1) * 128],
                                         hsB[j][:, k, BL:(S + 1) * BL],
                                         start=(k == 0), stop=(k == KD - 1))
                    nc.vector.tensor_tensor(h_sb[:, dm, dyn(col0, CC)],
                                            h_sb[:, dm, dyn(col0, CC)], po[:],
                                            ALU.add)
                # carry state to column 0, masked by gamma (0 resets at chunk 0)
                nc.vector.tensor_scalar(hsT[j][:, :, 0:BL],
                                        hsT[j][:, :, S * BL:(S + 1) * BL],
                                        gam_sb[:, j, dyn(it, 1)], None, ALU.mult)
                nc.vector.tensor_scalar(hsB[j][:, :, 0:BL],
                                        hsT[j][:, :, 0:BL],
                                        1.0, None, ALU.mult)

    # ================= final LN + logits =================
    WL = min(128, CC)
    hn2 = persist.tile([128, KD, CC], FP)
    for ec in range(c.NTOK // CC):
        layer_norm_chunk(c.PAD * CC + ec * CC, CC, h_sb, hn2)
        for t4 in range(CC // WL):
            pl = ps_proj.tile([128, c.V], FP, tag="px", bufs=2)
            for k in range(KD):
                nc.tensor.matmul(pl[:WL], hn2[:, k, t4 * WL:(t4 + 1) * WL],
                                 et_sb[:, k, :], start=(k == 0), stop=False)
            nc.tensor.matmul(pl[:WL], ones_col[:, 0:WL], bv_sb[:], start=False,
                             stop=True)
            out_sb = sb.tile([128, c.V], FP, tag="osb")
            nc.vector.tensor_copy(out_sb[:WL], pl[:WL])
            r0 = ec * CC + t4 * WL
            nc.sync.dma_start(logits[r0:r0 + WL, :], out_sb[:WL])


# ======================= host side =======================

def _pack_lhsT(m, kchunks, dtype=np.float32):
    # m: [K, J] with K = kchunks*128  ->  [128, kchunks, J]
    K, J = m.shape
    assert K == kchunks * 128
    return np.ascontiguousarray(m.reshape(kchunks, 128, J).transpose(1, 0, 2),
                                dtype=dtype)


def prep_inputs(inputs, cfg: Cfg):
    import ml_dtypes
    c = cfg
    f8 = np.float64
    sdt = ml_dtypes.bfloat16
    x = np.asarray(inputs["x"])
    emb = np.asarray(inputs["embedding"], f8)
    ln_g = np.asarray(inputs["ln_gamma"], f8)
    ln_b = np.asarray(inputs["ln_beta"], f8)
    Win = np.asarray(inputs["Win"], f8)
    W_zr = np.asarray(inputs["W_zr"], f8)
    U_zr = np.asarray(inputs["U_zr"], f8)
    W_h = np.asarray(inputs["W_h"], f8)
    U_h = np.asarray(inputs["U_h"], f8)
    b_zr = np.asarray(inputs["b_zr"], f8)
    b_h = np.asarray(inputs["b_h"], f8)
    Wout = np.asarray(inputs["Wout"], f8)
    ng = np.asarray(inputs["norm_gamma"], f8)
    nb = np.asarray(inputs["norm_beta"], f8)

    shared = {}
    L = c.DEPTH
    shared["UzrT_all"] = np.stack([_pack_lhsT(U_zr[l].T, c.KD, sdt) for l in range(L)])
    shared["UhT_all"] = np.stack([_pack_lhsT(U_h[l].T, c.KD, sdt) for l in range(L)])
    wzr_l, wh_l, bzr_l, bh_l, wo_l = [], [], [], [], []
    for l in range(L):
        Wzr_eff = W_zr[l] @ Win[l]                     # [2DI, D]
        bzr_eff = Wzr_eff @ ln_b[l] + b_zr[l]
        Wzr_eff = Wzr_eff * ln_g[l][None, :]
        Wh_eff = W_h[l] @ Win[l]
        bh_eff = Wh_eff @ ln_b[l] + b_h[l]
        Wh_eff = Wh_eff * ln_g[l][None, :]
        wzr_l.append(_pack_lhsT(Wzr_eff.T, c.KD, sdt))
        wh_l.append(_pack_lhsT(Wh_eff.T, c.KD, sdt))
        bzr_l.append(np.ascontiguousarray(
            bzr_eff.reshape(c.MZR, 128).T, dtype=np.float32))
        bh_l.append(np.ascontiguousarray(
            bh_eff.reshape(c.MH, 128).T, dtype=np.float32))
        wo_l.append(_pack_lhsT(Wout[l].T, c.KD, sdt))
    shared["WzrT_all"] = np.stack(wzr_l)
    shared["WhT_all"] = np.stack(wh_l)
    shared["bzr_all"] = np.stack(bzr_l)
    shared["bh_all"] = np.stack(bh_l)
    shared["WoT_all"] = np.stack(wo_l)
    shared["E_lhsT"] = np.ascontiguousarray(
        emb.reshape(c.KV, 128, c.D).transpose(1, 0, 2), dtype=sdt)
    shared["ET_rhs"] = _pack_lhsT((emb * ng[None, :]).T, c.KD)
    shared["bv_row"] = np.ascontiguousarray((emb @ nb)[None, :], dtype=np.float32)
    shared["iota2"] = np.ascontiguousarray(
        (np.arange(128)[:, None] + 128 * np.arange(c.KV)[None, :]), dtype=np.float32)
    shared["ones_col"] = np.ones((1, 128), np.float32)
    shared["ones_k"] = np.ones((128, 1), np.float32)
    # carry mask: the masked carry at end of iteration `it` feeds chunk
    # it+1-j, which must start from h0=0 when it+1-j <= 0, i.e. keep iff it>=j
    gam = np.zeros((128, c.W, c.NITER), np.float32)
    for j in range(c.W):
        gam[:, j, :] = (np.arange(c.NITER)[None, :] >= j).astype(np.float32)
    shared["gam"] = gam

    in_maps = []
    for core in range(c.n_cores):
        xc = x[core * c.BL:(core + 1) * c.BL, :]        # [BL, T]
        x_tb = np.ascontiguousarray(xc.T.reshape(1, -1), dtype=np.float32)
        m = dict(shared)
        m["x_tb"] = x_tb
        in_maps.append(m)
    return in_maps, shared


def declare_tensors(nc, cfg: Cfg, shared):
    c = cfg
    ins = {}
    ins["x_tb"] = nc.dram_tensor("x_tb", [1, c.NTOK], FP, kind="ExternalInput").ap()
    for name, arr in shared.items():
        if name == "x_tb":
            continue
        dt = mybir.dt.from_np(arr.dtype)
        ins[name] = nc.dram_tensor(name, list(arr.shape), dt, kind="ExternalInput").ap()
    outs = {}
    outs["logits"] = nc.dram_tensor("logits", [c.NTOK, c.V], FP,
                                    kind="ExternalOutput").ap()
    return outs, ins


_CACHE = {}


def build_program(cfg: Cfg, shared, enable_asserts=False):
    key = (cfg.DEPTH, cfg.T, cfg.S, cfg.U, cfg.W, cfg.n_cores)
    if key in _CACHE:
        return _CACHE[key]
    nc = bacc.Bacc("TRN2", target_bir_lowering=False, debug=False,
                   enable_asserts=enable_asserts, num_devices=cfg.n_cores)
    outs, ins = declare_tensors(nc, cfg, shared)
    with tile.TileContext(nc) as tc:
        with ExitStack() as ctx:
            build_kernel(ctx, tc, outs, ins, cfg)
    nc.compile()
    _CACHE[key] = nc
    return nc


def kernel(**inputs) -> np.ndarray:
    cfg = Cfg()
    in_maps, shared = prep_inputs(inputs, cfg)
    nc = build_program(cfg, shared)
    res = run_bass_kernel_spmd(nc, in_maps, core_ids=list(range(cfg.n_cores)))
    outs = []
    for core in range(cfg.n_cores):
        lg = res.results[core]["logits"]               # [NTOK, V], token = t*BL+b
        lg = lg.reshape(cfg.T, cfg.BL, cfg.V).transpose(1, 0, 2)
        outs.append(lg)
    return np.ascontiguousarray(np.concatenate(outs, axis=0), dtype=np.float32)


if __name__ == "__main__":
    # smoke test with random data
    rng = np.random.default_rng(0)
    cfg = Cfg()
    ins = dict(
        x=rng.integers(0, 256, size=(16, 2048)),
        embedding=rng.normal(size=(256, 512)).astype(np.float32) * 0.02,
        ln_gamma=np.ones((12, 512), np.float32),
        ln_beta=np.zeros((12, 512), np.float32),
        Win=rng.normal(size=(12, 512, 512)).astype(np.float32) * 0.02,
        W_zr=rng.normal(size=(12, 1024, 512)).astype(np.float32) * 0.02,
        U_zr=rng.normal(size=(12, 1024, 512)).astype(np.float32) * 0.04,
        W_h=rng.normal(size=(12, 512, 512)).astype(np.float32) * 0.02,
        U_h=rng.normal(size=(12, 512, 512)).astype(np.float32) * 0.04,
        b_zr=np.zeros((12, 1024), np.float32),
        b_h=np.zeros((12, 512), np.float32),
        Wout=rng.normal(size=(12, 512, 512)).astype(np.float32) * 0.02,
        norm_gamma=np.ones((512,), np.float32),
        norm_beta=np.zeros((512,), np.float32),
    )
    out = kernel(**ins)
    print(out.shape, out.dtype, np.abs(out).max())
